# revision 2
# baseline (speedup 1.0000x reference)
"""Per-sample dynamic 3x3 convolution (B=16, C=128, 64x64, pad 1) on 8
Trainium2 NeuronCores.

Sharding: pure data parallel -- batch 16 -> 2 samples per core, no
cross-core communication.

Device kernel (per core, per sample): 1-D Winograd F(2,3) along H on top
of an implicit-GEMM conv in bf16, which cuts tensor-engine MACs 1.5x vs
the direct 9-tap form (96 instead of 144 FD=512 matmuls per core):

  - host packs the G-transformed weights Ug[r] (r=0..3) as 12 lhsT
    slices [ci, co] per sample, and the zero-padded image split by row
    PARITY (even plane + odd plane) so every input-transform operand is
    a flat contiguous bf16 run (guaranteed DVE 2x mode).
  - per chunk of 8 row-pairs, VectorE builds the B^T row-planes
      A0 = e[i] - e[i+1],  A1 = o[i] + e[i+1],
      A2 = e[i+1] - o[i],  A3 = o[i] - o[i+1]
    (e/o = even/odd padded-row planes) with 4 contiguous TTs.
  - TensorE accumulates M_r = sum_dx Ug[r,dx]^T A_r[:, :, dx:dx+64]
    into 4 PSUM banks (FD=512); chunk-pairs run r={0,1} then r={2,3}
    phases with consecutive same-weight matmuls so redundant LDWEIGHTS
    cost is minimized and only 4 banks are live per phase.
  - ScalarE evacuates M0/M1/M2 to bf16 SBUF; VectorE combines
      Y_even = M0+M1+M2,  Y_odd = M1-M2-M3
    (M3 read straight from PSUM) and rows are stored as separate
    even/odd planes that the host re-interleaves.
  - DMA: sync ring carries sample 0 features + sample 1 stores; scalar
    ring carries weights (in phase-deadline order), sample 1 features,
    sample 0 stores.  All input DMAs are issued before any store so the
    FIFO rings can't head-of-line-block loads behind compute-gated
    stores.

Measured end-to-end rel l2 err ~4.7e-3 (bf16 data, fp32 PSUM accum).
"""

from contextlib import ExitStack

import numpy as np

B = 16
N_CORES = 8
BPC = B // N_CORES  # samples per core
CI = 128
CO = 128
H = W = 64
KS = 3
PADW = W + 2
PADH = H + 2
NPIX = H * W
NI = H // 2           # 32 output row-pairs
IPC = 8               # row-pairs per chunk
NCHUNK_W = NI // IPC  # 4 chunks per sample
WCOLS = 4 * KS * CO   # 12 weight slices of [ci, co]

_CACHE = {}


def _build_conv():
    import concourse.tile as tile
    from concourse import bacc, mybir

    F32 = mybir.dt.float32
    BF16 = mybir.dt.bfloat16
    ADD = mybir.AluOpType.add
    SUB = mybir.AluOpType.subtract

    nc = bacc.Bacc("TRN2", target_bir_lowering=False, debug=False,
                   num_devices=N_CORES)
    feats = nc.dram_tensor("features", [BPC, CI, PADH * PADW], BF16,
                           kind="ExternalInput").ap()
    wts = nc.dram_tensor("weights", [BPC, CI, WCOLS], BF16,
                         kind="ExternalInput").ap()
    out = nc.dram_tensor("out", [BPC, CO, NPIX], BF16,
                         kind="ExternalOutput").ap()

    with tile.TileContext(nc) as tc:
        with ExitStack() as ctx:
            xpool = ctx.enter_context(tc.tile_pool(name="xpad", bufs=2))
            wpool = ctx.enter_context(tc.tile_pool(name="wts", bufs=2))
            opool = ctx.enter_context(tc.tile_pool(name="outb", bufs=4))
            apool = ctx.enter_context(tc.tile_pool(name="aplanes", bufs=2))
            mpool = ctx.enter_context(tc.tile_pool(name="mcopy", bufs=4))
            pspool = ctx.enter_context(
                tc.tile_pool(name="psum", bufs=8, space="PSUM"))

            wt_tiles, xp_tiles, a_tiles = {}, {}, {}
            for b in range(BPC):
                wt_tiles[b] = wpool.tile([CI, WCOLS], BF16, tag="wt",
                                         name=f"wt{b}")
            # scalar ring in deadline order: sample-0 phase-A weights
            # first, then the rest, then sample-1 weights (its features
            # follow below)
            nc.scalar.dma_start(wt_tiles[0][:, :6 * CO], wts[0][:, :6 * CO])
            nc.scalar.dma_start(wt_tiles[0][:, 6 * CO:], wts[0][:, 6 * CO:])
            nc.scalar.dma_start(wt_tiles[1][:], wts[1])

            HH = PADH // 2  # 33 padded rows per parity plane
            pb = [0, 9, 17, 25, HH]
            for b in range(BPC):
                xp = xpool.tile([CI, PADH * PADW], BF16, tag="xp",
                                name=f"xp{b}")
                xpv = xp[:].rearrange("p (par h w) -> p par h w", par=2,
                                      w=PADW)
                fv = feats[b].rearrange("p (par h w) -> p par h w", par=2,
                                        w=PADW)
                for s0, s1 in zip(pb[:-1], pb[1:]):
                    eng = nc.sync if b == 0 else nc.scalar
                    for par in range(2):
                        eng.dma_start(xpv[:, par, s0:s1, :],
                                      fv[:, par, s0:s1, :])
                xp_tiles[b] = xp
                a_tiles[b] = apool.tile([CI, 4 * NI * PADW], BF16,
                                        tag="at", name=f"at{b}")

            def input_tts(b, i0, ni):
                """A-plane TTs for row-pairs [i0, i0+ni): all operands
                flat contiguous bf16 -> DVE 2x mode."""
                xpf = xp_tiles[b][:].rearrange("p (par q) -> p par q",
                                               par=2)
                d0 = xpf[:, 0, i0 * PADW:(i0 + ni) * PADW]
                d1 = xpf[:, 1, i0 * PADW:(i0 + ni) * PADW]
                d2 = xpf[:, 0, (i0 + 1) * PADW:(i0 + ni + 1) * PADW]
                d3 = xpf[:, 1, (i0 + 1) * PADW:(i0 + ni + 1) * PADW]
                avf = a_tiles[b][:].rearrange("p (r q) -> p r q", r=4)
                nc.vector.tensor_tensor(
                    avf[:, 0, i0 * PADW:(i0 + ni) * PADW], d0, d2, op=SUB)
                nc.vector.tensor_tensor(
                    avf[:, 1, i0 * PADW:(i0 + ni) * PADW], d1, d2, op=ADD)
                nc.vector.tensor_tensor(
                    avf[:, 2, i0 * PADW:(i0 + ni) * PADW], d2, d1, op=SUB)
                nc.vector.tensor_tensor(
                    avf[:, 3, i0 * PADW:(i0 + ni) * PADW], d1, d3, op=SUB)

            for b in range(BPC):
                wt = wt_tiles[b]
                av = a_tiles[b][:].rearrange("p (r i u) -> p r i u", r=4,
                                             u=PADW)
                for p in range(NCHUNK_W // 2):
                    cs = (2 * p, 2 * p + 1)
                    for c in cs:
                        input_tts(b, 8 * c, 8)

                    def mm_phase(rs):
                        ps = {}
                        for r in rs:
                            for c in cs:
                                ps[(r, c)] = pspool.tile(
                                    [CO, IPC * W], F32, tag="ps",
                                    name=f"ps{b}_{r}_{c}")
                            for dx in range(KS):
                                j = r * KS + dx
                                # chunk-pair shares each weight load
                                for c in cs:
                                    rhs = av[:, r, 8 * c:8 * c + 8,
                                             dx:dx + W]
                                    nc.tensor.matmul(
                                        ps[(r, c)][:],
                                        wt[:, j * CO:(j + 1) * CO], rhs,
                                        start=(dx == 0),
                                        stop=(dx == KS - 1))
                        return ps

                    psA = mm_phase((0, 1))
                    cp1, s = {}, {}
                    for c in cs:
                        c0t = mpool.tile([CO, IPC * W], BF16, tag="cp0",
                                         name=f"cp0_{b}_{c}")
                        cp1[c] = mpool.tile([CO, IPC * W], BF16,
                                            tag="cp1", name=f"cp1_{b}_{c}")
                        nc.scalar.copy(c0t[:], psA[(0, c)][:])
                        nc.scalar.copy(cp1[c][:], psA[(1, c)][:])
                        s[c] = mpool.tile([CO, IPC * W], BF16, tag="s",
                                          name=f"s_{b}_{c}")
                        nc.vector.tensor_tensor(s[c][:], c0t[:],
                                                cp1[c][:], op=ADD)
                    psB = mm_phase((2, 3))
                    for c in cs:
                        cp2 = mpool.tile([CO, IPC * W], BF16, tag="cp2",
                                         name=f"cp2_{b}_{c}")
                        nc.scalar.copy(cp2[:], psB[(2, c)][:])
                        ye = opool.tile([CO, IPC * W], BF16, tag="ye",
                                        name=f"ye_{b}_{c}")
                        d = mpool.tile([CO, IPC * W], BF16, tag="d",
                                       name=f"d_{b}_{c}")
                        yo = opool.tile([CO, IPC * W], BF16, tag="yo",
                                        name=f"yo_{b}_{c}")
                        nc.vector.tensor_tensor(ye[:], s[c][:], cp2[:],
                                                op=ADD)
                        nc.vector.tensor_tensor(d[:], cp1[c][:], cp2[:],
                                                op=SUB)
                        nc.vector.tensor_tensor(yo[:], d[:],
                                                psB[(3, c)][:], op=SUB)
                        off = c * IPC * W
                        eng = nc.scalar if b == 0 else nc.sync
                        eng.dma_start(out[b][:, off:off + IPC * W], ye[:])
                        eng.dma_start(
                            out[b][:, NPIX // 2 + off:
                                   NPIX // 2 + off + IPC * W], yo[:])
    nc.compile()
    return nc


def _host_pack_weights(dynamic_kernel):
    """G-transform along dy -> [B, CI, 4*3*CO] bf16; slice (r*3+dx) is
    the lhsT of Ug[r, :, :, dx]."""
    import ml_dtypes
    w = np.asarray(dynamic_kernel).astype(np.float32)  # [B, CO, CI, 3, 3]
    ug = np.empty((B, 4, CO, CI, KS), np.float32)
    ug[:, 0] = w[:, :, :, 0, :]
    ug[:, 1] = 0.5 * (w[:, :, :, 0, :] + w[:, :, :, 1, :] + w[:, :, :, 2, :])
    ug[:, 2] = 0.5 * (w[:, :, :, 0, :] - w[:, :, :, 1, :] + w[:, :, :, 2, :])
    ug[:, 3] = w[:, :, :, 2, :]
    ug = np.ascontiguousarray(ug.transpose(0, 3, 1, 4, 2))
    return ug.reshape(B, CI, WCOLS).astype(ml_dtypes.bfloat16)


def _host_pad_features(features):
    """Zero-pad to 66x66 and split by row parity: [B, CI, 2*33*66] bf16
    (even padded rows first, then odd)."""
    import ml_dtypes
    xp = np.zeros((B, CI, PADH, PADW), ml_dtypes.bfloat16)
    xp[:, :, 1:H + 1, 1:W + 1] = np.asarray(features).astype(
        ml_dtypes.bfloat16)
    xpe = xp[:, :, 0::2, :]
    xpo = xp[:, :, 1::2, :]
    return np.concatenate(
        [xpe.reshape(B, CI, -1), xpo.reshape(B, CI, -1)], axis=2)


def _host_unpack(got):
    """[B, CO, NPIX] (even-row plane then odd-row plane) ->
    [B, CO, H, W] fp32."""
    g = np.asarray(got).astype(np.float32).reshape(B, CO, 2, NI, W)
    o = np.empty((B, CO, H, W), np.float32)
    o[:, :, 0::2, :] = g[:, :, 0]
    o[:, :, 1::2, :] = g[:, :, 1]
    return o


def kernel(features, dynamic_kernel):
    """features (16,128,64,64) f32, dynamic_kernel (16,128,128,3,3) f32
    -> (16,128,64,64) f32."""
    from concourse.bass_utils import run_bass_kernel_spmd

    features = np.asarray(features)
    dynamic_kernel = np.asarray(dynamic_kernel)

    if "nc" not in _CACHE:
        _CACHE["nc"] = _build_conv()
    nc = _CACHE["nc"]

    f_padded = _host_pad_features(features)
    w_packed = _host_pack_weights(dynamic_kernel)
    in_maps = [{"features": f_padded[BPC * c:BPC * (c + 1)],
                "weights": w_packed[BPC * c:BPC * (c + 1)]}
               for c in range(N_CORES)]

    import time as _time
    last_err = None
    for attempt in range(4):  # transient NRT/device errors: retry
        try:
            res = run_bass_kernel_spmd(nc, in_maps,
                                       core_ids=list(range(N_CORES)))
            break
        except Exception as e:  # noqa: BLE001
            last_err = e
            # give the terminal time to recover a wedged core before
            # the next attempt (immediate retries hit the same state)
            _time.sleep(5 * (attempt + 1))
    else:
        raise last_err

    got = np.concatenate([res.results[c]["out"] for c in range(N_CORES)],
                         axis=0)
    return _host_unpack(got)


# revision 3
# speedup vs baseline: 1.0287x; 1.0287x over previous
"""Per-sample dynamic 3x3 convolution (B=16, C=128, 64x64, pad 1) on 8
Trainium2 NeuronCores.

Sharding: pure data parallel -- batch 16 -> 2 samples per core, no
cross-core communication.

Device kernel (per core, per sample): 1-D Winograd F(2,3) along H on top
of an implicit-GEMM conv in bf16, which cuts tensor-engine MACs 1.5x vs
the direct 9-tap form (96 instead of 144 FD=512 matmuls per core):

  - host packs the G-transformed weights Ug[r] (r=0..3) as 12 lhsT
    slices [ci, co] per sample, and the zero-padded image split by row
    PARITY (even plane + odd plane) so every input-transform operand is
    a flat contiguous bf16 run (guaranteed DVE 2x mode).
  - per chunk of 8 row-pairs, VectorE builds the B^T row-planes
      A0 = e[i] - e[i+1],  A1 = o[i] + e[i+1],
      A2 = e[i+1] - o[i],  A3 = o[i] - o[i+1]
    (e/o = even/odd padded-row planes) with 4 contiguous TTs.
  - TensorE accumulates M_r = sum_dx Ug[r,dx]^T A_r[:, :, dx:dx+64]
    into 4 PSUM banks (FD=512); chunk-pairs run r={0,1} then r={2,3}
    phases with consecutive same-weight matmuls so redundant LDWEIGHTS
    cost is minimized and only 4 banks are live per phase.
  - ScalarE evacuates M0/M1/M2 to bf16 SBUF; VectorE combines
      Y_even = M0+M1+M2,  Y_odd = M1-M2-M3
    (M3 read straight from PSUM) and rows are stored as separate
    even/odd planes that the host re-interleaves.
  - DMA: sync ring carries sample 0 features + sample 1 stores; scalar
    ring carries weights (in phase-deadline order), sample 1 features,
    sample 0 stores.  All input DMAs are issued before any store so the
    FIFO rings can't head-of-line-block loads behind compute-gated
    stores.

Measured end-to-end rel l2 err ~4.7e-3 (bf16 data, fp32 PSUM accum).
"""

from contextlib import ExitStack

import numpy as np

B = 16
N_CORES = 8
BPC = B // N_CORES  # samples per core
CI = 128
CO = 128
H = W = 64
KS = 3
PADW = W + 2
PADH = H + 2
NPIX = H * W
NI = H // 2           # 32 output row-pairs
IPC = 8               # row-pairs per chunk
NCHUNK_W = NI // IPC  # 4 chunks per sample
WCOLS = 4 * KS * CO   # 12 weight slices of [ci, co]

_CACHE = {}


def _build_conv():
    import concourse.tile as tile
    from concourse import bacc, mybir

    F32 = mybir.dt.float32
    BF16 = mybir.dt.bfloat16
    ADD = mybir.AluOpType.add
    SUB = mybir.AluOpType.subtract

    nc = bacc.Bacc("TRN2", target_bir_lowering=False, debug=False,
                   num_devices=N_CORES)
    feats = nc.dram_tensor("features", [BPC, CI, PADH * PADW], BF16,
                           kind="ExternalInput").ap()
    wts = nc.dram_tensor("weights", [BPC, CI, WCOLS], BF16,
                         kind="ExternalInput").ap()
    out = nc.dram_tensor("out", [BPC, CO, NPIX], BF16,
                         kind="ExternalOutput").ap()

    with tile.TileContext(nc) as tc:
        with ExitStack() as ctx:
            xpool = ctx.enter_context(tc.tile_pool(name="xpad", bufs=2))
            wpool = ctx.enter_context(tc.tile_pool(name="wts", bufs=2))
            opool = ctx.enter_context(tc.tile_pool(name="outb", bufs=4))
            apool = ctx.enter_context(tc.tile_pool(name="aplanes", bufs=2))
            mpool = ctx.enter_context(tc.tile_pool(name="mcopy", bufs=4))
            pspool = ctx.enter_context(
                tc.tile_pool(name="psum", bufs=8, space="PSUM"))

            wt_tiles, xp_tiles, a_tiles = {}, {}, {}
            for b in range(BPC):
                wt_tiles[b] = wpool.tile([CI, WCOLS], BF16, tag="wt",
                                         name=f"wt{b}")
            # scalar ring in deadline order: the (r0,dx0) weight slice
            # alone first (the very first matmul needs only it plus the
            # A0 plane, which in turn needs only the first even-parity
            # feature slice), then the rest of sample-0's phase-A
            # weights, sample-0's phase-B weights, sample-1 weights
            # (its features follow below)
            nc.scalar.dma_start(wt_tiles[0][:, :CO], wts[0][:, :CO])
            nc.scalar.dma_start(wt_tiles[0][:, CO:6 * CO],
                                wts[0][:, CO:6 * CO])
            nc.scalar.dma_start(wt_tiles[0][:, 6 * CO:], wts[0][:, 6 * CO:])
            nc.scalar.dma_start(wt_tiles[1][:], wts[1])

            HH = PADH // 2  # 33 padded rows per parity plane
            pb = [0, 9, 17, 25, HH]
            for b in range(BPC):
                xp = xpool.tile([CI, PADH * PADW], BF16, tag="xp",
                                name=f"xp{b}")
                xpv = xp[:].rearrange("p (par h w) -> p par h w", par=2,
                                      w=PADW)
                fv = feats[b].rearrange("p (par h w) -> p par h w", par=2,
                                        w=PADW)
                for s0, s1 in zip(pb[:-1], pb[1:]):
                    eng = nc.sync if b == 0 else nc.scalar
                    for par in range(2):
                        eng.dma_start(xpv[:, par, s0:s1, :],
                                      fv[:, par, s0:s1, :])
                xp_tiles[b] = xp
                a_tiles[b] = apool.tile([CI, 4 * NI * PADW], BF16,
                                        tag="at", name=f"at{b}")

            def input_tts(b, i0, ni):
                """A-plane TTs for row-pairs [i0, i0+ni): all operands
                flat contiguous bf16 -> DVE 2x mode."""
                xpf = xp_tiles[b][:].rearrange("p (par q) -> p par q",
                                               par=2)
                d0 = xpf[:, 0, i0 * PADW:(i0 + ni) * PADW]
                d1 = xpf[:, 1, i0 * PADW:(i0 + ni) * PADW]
                d2 = xpf[:, 0, (i0 + 1) * PADW:(i0 + ni + 1) * PADW]
                d3 = xpf[:, 1, (i0 + 1) * PADW:(i0 + ni + 1) * PADW]
                avf = a_tiles[b][:].rearrange("p (r q) -> p r q", r=4)
                nc.vector.tensor_tensor(
                    avf[:, 0, i0 * PADW:(i0 + ni) * PADW], d0, d2, op=SUB)
                nc.vector.tensor_tensor(
                    avf[:, 1, i0 * PADW:(i0 + ni) * PADW], d1, d2, op=ADD)
                nc.vector.tensor_tensor(
                    avf[:, 2, i0 * PADW:(i0 + ni) * PADW], d2, d1, op=SUB)
                nc.vector.tensor_tensor(
                    avf[:, 3, i0 * PADW:(i0 + ni) * PADW], d1, d3, op=SUB)

            for b in range(BPC):
                wt = wt_tiles[b]
                av = a_tiles[b][:].rearrange("p (r i u) -> p r i u", r=4,
                                             u=PADW)
                for p in range(NCHUNK_W // 2):
                    cs = (2 * p, 2 * p + 1)
                    for c in cs:
                        input_tts(b, 8 * c, 8)

                    def mm_phase(rs):
                        ps = {}
                        for r in rs:
                            for c in cs:
                                ps[(r, c)] = pspool.tile(
                                    [CO, IPC * W], F32, tag="ps",
                                    name=f"ps{b}_{r}_{c}")
                            for dx in range(KS):
                                j = r * KS + dx
                                # chunk-pair shares each weight load
                                for c in cs:
                                    rhs = av[:, r, 8 * c:8 * c + 8,
                                             dx:dx + W]
                                    nc.tensor.matmul(
                                        ps[(r, c)][:],
                                        wt[:, j * CO:(j + 1) * CO], rhs,
                                        start=(dx == 0),
                                        stop=(dx == KS - 1))
                        return ps

                    psA = mm_phase((0, 1))
                    cp1, s = {}, {}
                    for c in cs:
                        c0t = mpool.tile([CO, IPC * W], BF16, tag="cp0",
                                         name=f"cp0_{b}_{c}")
                        cp1[c] = mpool.tile([CO, IPC * W], BF16,
                                            tag="cp1", name=f"cp1_{b}_{c}")
                        nc.scalar.copy(c0t[:], psA[(0, c)][:])
                        nc.scalar.copy(cp1[c][:], psA[(1, c)][:])
                        s[c] = mpool.tile([CO, IPC * W], BF16, tag="s",
                                          name=f"s_{b}_{c}")
                        nc.vector.tensor_tensor(s[c][:], c0t[:],
                                                cp1[c][:], op=ADD)
                    psB = mm_phase((2, 3))
                    for c in cs:
                        cp2 = mpool.tile([CO, IPC * W], BF16, tag="cp2",
                                         name=f"cp2_{b}_{c}")
                        nc.scalar.copy(cp2[:], psB[(2, c)][:])
                        ye = opool.tile([CO, IPC * W], BF16, tag="ye",
                                        name=f"ye_{b}_{c}")
                        d = mpool.tile([CO, IPC * W], BF16, tag="d",
                                       name=f"d_{b}_{c}")
                        yo = opool.tile([CO, IPC * W], BF16, tag="yo",
                                        name=f"yo_{b}_{c}")
                        nc.vector.tensor_tensor(ye[:], s[c][:], cp2[:],
                                                op=ADD)
                        nc.vector.tensor_tensor(d[:], cp1[c][:], cp2[:],
                                                op=SUB)
                        nc.vector.tensor_tensor(yo[:], d[:],
                                                psB[(3, c)][:], op=SUB)
                        off = c * IPC * W
                        eng = nc.scalar if b == 0 else nc.sync
                        eng.dma_start(out[b][:, off:off + IPC * W], ye[:])
                        eng.dma_start(
                            out[b][:, NPIX // 2 + off:
                                   NPIX // 2 + off + IPC * W], yo[:])
    nc.compile()
    return nc


def _host_pack_weights(dynamic_kernel):
    """G-transform along dy -> [B, CI, 4*3*CO] bf16; slice (r*3+dx) is
    the lhsT of Ug[r, :, :, dx]."""
    import ml_dtypes
    w = np.asarray(dynamic_kernel).astype(np.float32)  # [B, CO, CI, 3, 3]
    ug = np.empty((B, 4, CO, CI, KS), np.float32)
    ug[:, 0] = w[:, :, :, 0, :]
    ug[:, 1] = 0.5 * (w[:, :, :, 0, :] + w[:, :, :, 1, :] + w[:, :, :, 2, :])
    ug[:, 2] = 0.5 * (w[:, :, :, 0, :] - w[:, :, :, 1, :] + w[:, :, :, 2, :])
    ug[:, 3] = w[:, :, :, 2, :]
    ug = np.ascontiguousarray(ug.transpose(0, 3, 1, 4, 2))
    return ug.reshape(B, CI, WCOLS).astype(ml_dtypes.bfloat16)


def _host_pad_features(features):
    """Zero-pad to 66x66 and split by row parity: [B, CI, 2*33*66] bf16
    (even padded rows first, then odd)."""
    import ml_dtypes
    xp = np.zeros((B, CI, PADH, PADW), ml_dtypes.bfloat16)
    xp[:, :, 1:H + 1, 1:W + 1] = np.asarray(features).astype(
        ml_dtypes.bfloat16)
    xpe = xp[:, :, 0::2, :]
    xpo = xp[:, :, 1::2, :]
    return np.concatenate(
        [xpe.reshape(B, CI, -1), xpo.reshape(B, CI, -1)], axis=2)


def _host_unpack(got):
    """[B, CO, NPIX] (even-row plane then odd-row plane) ->
    [B, CO, H, W] fp32."""
    g = np.asarray(got).astype(np.float32).reshape(B, CO, 2, NI, W)
    o = np.empty((B, CO, H, W), np.float32)
    o[:, :, 0::2, :] = g[:, :, 0]
    o[:, :, 1::2, :] = g[:, :, 1]
    return o


def kernel(features, dynamic_kernel):
    """features (16,128,64,64) f32, dynamic_kernel (16,128,128,3,3) f32
    -> (16,128,64,64) f32."""
    from concourse.bass_utils import run_bass_kernel_spmd

    features = np.asarray(features)
    dynamic_kernel = np.asarray(dynamic_kernel)

    if "nc" not in _CACHE:
        _CACHE["nc"] = _build_conv()
    nc = _CACHE["nc"]

    f_padded = _host_pad_features(features)
    w_packed = _host_pack_weights(dynamic_kernel)
    in_maps = [{"features": f_padded[BPC * c:BPC * (c + 1)],
                "weights": w_packed[BPC * c:BPC * (c + 1)]}
               for c in range(N_CORES)]

    import time as _time
    last_err = None
    for attempt in range(4):  # transient NRT/device errors: retry
        try:
            res = run_bass_kernel_spmd(nc, in_maps,
                                       core_ids=list(range(N_CORES)))
            break
        except Exception as e:  # noqa: BLE001
            last_err = e
            # give the terminal time to recover a wedged core before
            # the next attempt (immediate retries hit the same state)
            _time.sleep(5 * (attempt + 1))
    else:
        raise last_err

    got = np.concatenate([res.results[c]["out"] for c in range(N_CORES)],
                         axis=0)
    return _host_unpack(got)
